# revision 2
# baseline (speedup 1.0000x reference)
"""Trainium2 Bass kernel for nn_AdditiveCouplingLayer.

y = x; y[:, 1::2] += MLP(x[:, 0::2])  with a 512->1024->1024->512 relu MLP.

Strategy: data-parallel over 8 NeuronCores (batch 65536 -> 8192/core),
weights replicated. Per core the MLP runs in "transposed activation"
space (features on partitions, batch on the free dim) so every matmul
uses the natural weight layout; x is transposed on the PE (128x128
blocks of the even columns) on the way in and the translation is
transposed back on the way out. Matmuls run in fp16 (1 cycle/row on the
PE vs 4 for fp32) with fp32 PSUM accumulation; weights are pre-cast to
fp16 on the host. MODE "f16x3" upgrades to near-fp32 precision via a
3-term hi/lo split (3x the matmul work).
"""

import os
import sys

sys.path.insert(0, "/opt/trn_rl_repo")

import numpy as np

B, D, F, H = 65536, 1024, 512, 1024
NCORES = 8
BPC = B // NCORES  # rows per core
TB = 512  # batch tile (matmul free dim)
NBT = BPC // TB  # batch tiles per core
MODE = os.environ.get("BASS_COUPLING_MODE", "f16")

_cache = {}


def _build(mode):
    import concourse.bacc as bacc
    import concourse.tile as tile
    import concourse.mybir as mybir

    dt = mybir.dt
    AF = mybir.ActivationFunctionType
    split = mode == "f16x3"

    nc = bacc.Bacc(
        "TRN2", target_bir_lowering=False, debug=False, num_devices=NCORES
    )

    x_d = nc.dram_tensor("x", [BPC, D], dt.float32, kind="ExternalInput").ap()
    w_d = {}
    for name, shape in (("w1", [F, H]), ("w2", [H, H]), ("w3", [H, F])):
        w_d[name] = nc.dram_tensor(name, shape, dt.float16, kind="ExternalInput").ap()
        if split:
            w_d[name + "l"] = nc.dram_tensor(
                name + "l", shape, dt.float16, kind="ExternalInput"
            ).ap()
    b1_d = nc.dram_tensor("b1", [H], dt.float32, kind="ExternalInput").ap()
    b2_d = nc.dram_tensor("b2", [H], dt.float32, kind="ExternalInput").ap()
    b3_d = nc.dram_tensor("b3", [F], dt.float32, kind="ExternalInput").ap()
    id_d = nc.dram_tensor("ident", [128, 128], dt.float32, kind="ExternalInput").ap()
    y_d = nc.dram_tensor("y", [BPC, D], dt.float32, kind="ExternalOutput").ap()

    with tile.TileContext(nc) as tc:
        with (
            tc.tile_pool(name="wpool", bufs=1) as wpool,
            tc.tile_pool(name="xpool", bufs=2) as xpool,
            tc.tile_pool(name="ypool", bufs=2) as ypool,
            tc.tile_pool(name="mpool", bufs=2) as mpool,
            tc.tile_pool(name="hpool", bufs=2) as hpool,
            tc.tile_pool(name="tpool", bufs=2) as tpool,
            tc.tile_pool(name="pmm", bufs=3, space="PSUM") as pmm,
            tc.tile_pool(name="ptp", bufs=4, space="PSUM") as ptp,
        ):
            # --- resident weights/biases/identity ---
            def load_w(name, rows, cols):
                ts = []
                for k in range(rows // 128):
                    t = wpool.tile([128, cols], dt.float16, tag=f"{name}_{k}")
                    nc.sync.dma_start(t[:], w_d[name][k * 128 : (k + 1) * 128, :])
                    ts.append(t)
                return ts

            w1t = load_w("w1", F, H)
            w2t = load_w("w2", H, H)
            w3t = load_w("w3", H, F)
            if split:
                w1l = load_w("w1l", F, H)
                w2l = load_w("w2l", H, H)
                w3l = load_w("w3l", H, F)

            def load_b(name, ap, n):
                t = wpool.tile([128, n // 128], dt.float32, tag=name)
                nc.sync.dma_start(t[:], ap.rearrange("(m p) -> p m", p=128))
                return t

            b1t = load_b("b1t", b1_d, H)
            b2t = load_b("b2t", b2_d, H)
            b3t = load_b("b3t", b3_d, F)
            ident = wpool.tile([128, 128], dt.float32, tag="ident")
            nc.sync.dma_start(ident[:], id_d[:])

            def mm_group(psum, pairs):
                n = len(pairs)
                for i, (lhsT, rhs) in enumerate(pairs):
                    nc.tensor.matmul(
                        psum[:], lhsT, rhs, start=(i == 0), stop=(i == n - 1)
                    )

            def layer(wt, wl, ins, ins_lo, bt, nout, act, out_dt, out_pool, oname):
                outs = []
                outs_lo = []
                nk = len(ins)
                for m in range(nout // 128):
                    p = pmm.tile([128, TB], dt.float32, tag="mm")
                    ms = slice(m * 128, (m + 1) * 128)
                    pairs = [(wt[k][:, ms], ins[k][:]) for k in range(nk)]
                    if split:
                        pairs += [(wt[k][:, ms], ins_lo[k][:]) for k in range(nk)]
                        pairs += [(wl[k][:, ms], ins[k][:]) for k in range(nk)]
                    mm_group(p, pairs)
                    o = out_pool.tile([128, TB], out_dt, tag=f"{oname}_{m}")
                    nc.scalar.activation(o[:], p[:], act, bias=bt[:, m : m + 1])
                    outs.append(o)
                    if split and out_dt == dt.float16:
                        # hi/lo split of the fp32 activation result:
                        # need fp32 master first; recompute via two-step
                        of = out_pool.tile([128, TB], dt.float32, tag=f"{oname}f_{m}")
                        nc.scalar.activation(of[:], p[:], act, bias=bt[:, m : m + 1])
                        ol = out_pool.tile([128, TB], dt.float16, tag=f"{oname}l_{m}")
                        nc.vector.tensor_sub(ol[:], of[:], o[:])
                        outs_lo.append(ol)
                return outs, outs_lo

            for bt_i in range(NBT):
                r0 = bt_i * TB
                # load x tile (4 chunks of 128 rows)
                xb = []
                for i in range(4):
                    t = xpool.tile([128, D], dt.float32, tag=f"x{i}")
                    nc.sync.dma_start(
                        t[:], x_d[r0 + i * 128 : r0 + (i + 1) * 128, :]
                    )
                    xb.append(t)

                # transpose masked (even) columns -> mT[j] [128 feat, TB batch]
                mT = []
                mTl = []
                for j in range(4):
                    t = mpool.tile([128, TB], dt.float16, tag=f"m{j}")
                    tf = (
                        mpool.tile([128, TB], dt.float32, tag=f"mf{j}")
                        if split
                        else None
                    )
                    tl = (
                        mpool.tile([128, TB], dt.float16, tag=f"ml{j}")
                        if split
                        else None
                    )
                    for i in range(4):
                        tp = ptp.tile([128, 128], dt.float32, tag="tp")
                        nc.tensor.transpose(
                            tp[:],
                            xb[i][:, 256 * j : 256 * (j + 1) : 2],
                            ident[:],
                        )
                        cs = slice(i * 128, (i + 1) * 128)
                        nc.vector.tensor_copy(t[:, cs], tp[:])
                        if split:
                            nc.scalar.copy(tf[:, cs], tp[:])
                    if split:
                        nc.vector.tensor_sub(tl[:], tf[:], t[:])
                        mTl.append(tl)
                    mT.append(t)

                h1, h1l = layer(
                    w1t, w1l if split else None, mT, mTl, b1t, H,
                    AF.Relu, dt.float16, hpool, "h1",
                )
                h2, h2l = layer(
                    w2t, w2l if split else None, h1, h1l, b2t, H,
                    AF.Relu, dt.float16, hpool, "h2",
                )
                tT, _ = layer(
                    w3t, w3l if split else None, h2, h2l, b3t, F,
                    AF.Identity, dt.float32, tpool, "t",
                )

                # assemble y: even cols copied, odd cols = x_odd + t^T
                yb = []
                for i in range(4):
                    t = ypool.tile([128, D], dt.float32, tag=f"y{i}")
                    nc.vector.tensor_copy(t[:, 0:D:2], xb[i][:, 0:D:2])
                    yb.append(t)
                for m in range(4):
                    for i in range(4):
                        tp = ptp.tile([128, 128], dt.float32, tag="tp")
                        nc.tensor.transpose(
                            tp[:], tT[m][:, i * 128 : (i + 1) * 128], ident[:]
                        )
                        osl = slice(256 * m + 1, 256 * (m + 1), 2)
                        nc.vector.tensor_add(yb[i][:, osl], xb[i][:, osl], tp[:])
                for i in range(4):
                    nc.sync.dma_start(
                        y_d[r0 + i * 128 : r0 + (i + 1) * 128, :], yb[i][:]
                    )

    nc.compile()
    return nc


def _get(mode):
    if mode not in _cache:
        _cache[mode] = _build(mode)
    return _cache[mode]


def _in_maps(x, W1, b1, W2, b2, W3, b3):
    split = MODE == "f16x3"

    def prep_w(w):
        hi = np.asarray(w, dtype=np.float32).astype(np.float16)
        if not split:
            return {"": hi}
        lo = (np.asarray(w, dtype=np.float32) - hi.astype(np.float32)).astype(
            np.float16
        )
        return {"": hi, "l": lo}

    ws = {}
    for name, w in (("w1", W1), ("w2", W2), ("w3", W3)):
        for suf, arr in prep_w(w).items():
            ws[name + suf] = arr

    common = dict(
        ws,
        b1=np.asarray(b1, np.float32),
        b2=np.asarray(b2, np.float32),
        b3=np.asarray(b3, np.float32),
        ident=np.eye(128, dtype=np.float32),
    )
    x = np.ascontiguousarray(np.asarray(x, np.float32))
    return [dict(common, x=x[c * BPC : (c + 1) * BPC]) for c in range(NCORES)]


def kernel(x, W1, b1, W2, b2, W3, b3):
    from concourse.bass_utils import run_bass_kernel_spmd

    nc = _get(MODE)
    res = run_bass_kernel_spmd(
        nc, _in_maps(x, W1, b1, W2, b2, W3, b3), core_ids=list(range(NCORES))
    )
    return np.concatenate([res.results[c]["y"] for c in range(NCORES)], axis=0)


# revision 3
# speedup vs baseline: 1.2265x; 1.2265x over previous
"""Trainium2 Bass kernel for nn_AdditiveCouplingLayer.

y = x; y[:, 1::2] += MLP(x[:, 0::2])  with a 512->1024->1024->512 relu MLP.

Strategy: data-parallel over 8 NeuronCores (batch 65536 -> 8192/core),
weights replicated. The MLP's first two layers run in "transposed
activation" space (features on partitions, batch on the free dim) so
every matmul uses the natural weight layout; the host supplies the
masked half of x pre-transposed and pre-cast to fp16. Layer 3 swaps the
matmul operand roles (h2 slice stationary, W3 moving) so the
translation comes out in natural [batch, feature] layout — no output
transpose needed. Matmuls run in fp16 (1 cycle/row on the PE vs 4 for
fp32) with fp32 PSUM accumulation; weights are pre-cast to fp16 on the
host. MODE "f16x3" upgrades to near-fp32 precision via a 3-term hi/lo
split (3x the matmul work).
"""

import os
import sys

sys.path.insert(0, "/opt/trn_rl_repo")

import numpy as np

B, D, F, H = 65536, 1024, 512, 1024
NCORES = 8
BPC = B // NCORES  # rows per core
TB = 512  # batch tile (matmul free dim)
NBT = BPC // TB  # batch tiles per core
MODE = os.environ.get("BASS_COUPLING_MODE", "f16")

_cache = {}


def _build(mode):
    import concourse.bacc as bacc
    import concourse.tile as tile
    import concourse.mybir as mybir

    dt = mybir.dt
    AF = mybir.ActivationFunctionType
    split = mode == "f16x3"

    nc = bacc.Bacc(
        "TRN2", target_bir_lowering=False, debug=False, num_devices=NCORES
    )

    x_d = nc.dram_tensor("x", [BPC, D], dt.float32, kind="ExternalInput").ap()
    mT_d = nc.dram_tensor("mT", [F, BPC], dt.float16, kind="ExternalInput").ap()
    if split:
        mTl_d = nc.dram_tensor("mTl", [F, BPC], dt.float16, kind="ExternalInput").ap()
    w_d = {}
    for name, shape in (("w1", [F, H]), ("w2", [H, H]), ("w3", [H, F])):
        w_d[name] = nc.dram_tensor(name, shape, dt.float16, kind="ExternalInput").ap()
        if split:
            w_d[name + "l"] = nc.dram_tensor(
                name + "l", shape, dt.float16, kind="ExternalInput"
            ).ap()
    b1_d = nc.dram_tensor("b1", [H], dt.float32, kind="ExternalInput").ap()
    b2_d = nc.dram_tensor("b2", [H], dt.float32, kind="ExternalInput").ap()
    b3r_d = nc.dram_tensor("b3rep", [128, F], dt.float32, kind="ExternalInput").ap()
    y_d = nc.dram_tensor("y", [BPC, D], dt.float32, kind="ExternalOutput").ap()

    with tile.TileContext(nc) as tc:
        with (
            tc.tile_pool(name="wpool", bufs=1) as wpool,
            tc.tile_pool(name="xpool", bufs=2) as xpool,
            tc.tile_pool(name="ypool", bufs=2) as ypool,
            tc.tile_pool(name="mpool", bufs=3) as mpool,
            tc.tile_pool(name="hpool", bufs=2) as hpool,
            tc.tile_pool(name="pmm", bufs=4, space="PSUM") as pmm,
        ):
            # --- resident weights/biases (issued after first mT DMAs via
            # program order; Tile tracks per-tile deps so compute starts as
            # soon as its own inputs land) ---
            def load_w(name, rows, cols):
                ts = []
                for k in range(rows // 128):
                    t = wpool.tile([128, cols], dt.float16, tag=f"{name}_{k}")
                    nc.sync.dma_start(t[:], w_d[name][k * 128 : (k + 1) * 128, :])
                    ts.append(t)
                return ts

            w1t = load_w("w1", F, H)
            w2t = load_w("w2", H, H)
            w3t = load_w("w3", H, F)
            if split:
                w1l = load_w("w1l", F, H)
                w2l = load_w("w2l", H, H)
                w3l = load_w("w3l", H, F)

            def load_b(name, ap, n):
                t = wpool.tile([128, n // 128], dt.float32, tag=name)
                nc.sync.dma_start(t[:], ap.rearrange("(m p) -> p m", p=128))
                return t

            b1t = load_b("b1t", b1_d, H)
            b2t = load_b("b2t", b2_d, H)
            b3rep = wpool.tile([128, F], dt.float32, tag="b3rep")
            nc.sync.dma_start(b3rep[:], b3r_d[:])

            def mm_group(psum, pairs):
                n = len(pairs)
                for i, (lhsT, rhs) in enumerate(pairs):
                    nc.tensor.matmul(
                        psum[:], lhsT, rhs, start=(i == 0), stop=(i == n - 1)
                    )

            def layer(wt, wl, ins, ins_lo, bt, nout, oname):
                """Transposed-space layer: out[m][feat128, TB] = relu(W.T@in + b)."""
                outs = []
                outs_lo = []
                nk = len(ins)
                for m in range(nout // 128):
                    p = pmm.tile([128, TB], dt.float32, tag="mm")
                    ms = slice(m * 128, (m + 1) * 128)
                    pairs = [(wt[k][:, ms], ins[k][:]) for k in range(nk)]
                    if split:
                        pairs += [(wt[k][:, ms], ins_lo[k][:]) for k in range(nk)]
                        pairs += [(wl[k][:, ms], ins[k][:]) for k in range(nk)]
                    mm_group(p, pairs)
                    o = hpool.tile([128, TB], dt.float16, tag=f"{oname}_{m}")
                    nc.scalar.activation(o[:], p[:], AF.Relu, bias=bt[:, m : m + 1])
                    outs.append(o)
                    if split:
                        of = hpool.tile([128, TB], dt.float32, tag=f"{oname}f_{m}")
                        nc.scalar.activation(
                            of[:], p[:], AF.Relu, bias=bt[:, m : m + 1]
                        )
                        ol = hpool.tile([128, TB], dt.float16, tag=f"{oname}l_{m}")
                        nc.vector.tensor_sub(ol[:], of[:], o[:])
                        outs_lo.append(ol)
                return outs, outs_lo

            for bt_i in range(NBT):
                r0 = bt_i * TB

                # masked^T fp16 tiles straight from DRAM (host pre-transposed)
                mT = []
                mTl = []
                for j in range(4):
                    t = mpool.tile([128, TB], dt.float16, tag=f"m{j}")
                    nc.sync.dma_start(
                        t[:], mT_d[j * 128 : (j + 1) * 128, r0 : r0 + TB]
                    )
                    mT.append(t)
                    if split:
                        tl = mpool.tile([128, TB], dt.float16, tag=f"ml{j}")
                        nc.sync.dma_start(
                            tl[:], mTl_d[j * 128 : (j + 1) * 128, r0 : r0 + TB]
                        )
                        mTl.append(tl)

                # x tile (natural layout, for the residual assembly)
                xb = []
                for i in range(4):
                    t = xpool.tile([128, D], dt.float32, tag=f"x{i}")
                    nc.sync.dma_start(
                        t[:], x_d[r0 + i * 128 : r0 + (i + 1) * 128, :]
                    )
                    xb.append(t)

                h1, h1l = layer(
                    w1t, w1l if split else None, mT, mTl, b1t, H, "h1"
                )
                h2, h2l = layer(
                    w2t, w2l if split else None, h1, h1l, b2t, H, "h2"
                )

                # y assembly: even cols copied; odd cols = x_odd + b3 + t
                yb = []
                for i in range(4):
                    t = ypool.tile([128, D], dt.float32, tag=f"y{i}")
                    nc.vector.tensor_copy(t[:, 0:D:2], xb[i][:, 0:D:2])
                    nc.vector.tensor_add(t[:, 1:D:2], xb[i][:, 1:D:2], b3rep[:])
                    yb.append(t)

                # layer 3 in natural layout: stationary = h2 batch-slice,
                # moving = W3 tile  ->  psum[batch128, F]
                for i in range(4):
                    p = pmm.tile([128, F], dt.float32, tag="mm")
                    bs = slice(i * 128, (i + 1) * 128)
                    pairs = [(h2[k][:, bs], w3t[k][:]) for k in range(8)]
                    if split:
                        pairs += [(h2l[k][:, bs], w3t[k][:]) for k in range(8)]
                        pairs += [(h2[k][:, bs], w3l[k][:]) for k in range(8)]
                    mm_group(p, pairs)
                    nc.vector.tensor_add(yb[i][:, 1:D:2], yb[i][:, 1:D:2], p[:])

                for i in range(4):
                    nc.sync.dma_start(
                        y_d[r0 + i * 128 : r0 + (i + 1) * 128, :], yb[i][:]
                    )

    nc.compile()
    return nc


def _get(mode):
    if mode not in _cache:
        _cache[mode] = _build(mode)
    return _cache[mode]


def _in_maps(x, W1, b1, W2, b2, W3, b3):
    split = MODE == "f16x3"

    def prep_w(w):
        hi = np.asarray(w, dtype=np.float32).astype(np.float16)
        if not split:
            return {"": hi}
        lo = (np.asarray(w, dtype=np.float32) - hi.astype(np.float32)).astype(
            np.float16
        )
        return {"": hi, "l": lo}

    ws = {}
    for name, w in (("w1", W1), ("w2", W2), ("w3", W3)):
        for suf, arr in prep_w(w).items():
            ws[name + suf] = arr

    common = dict(
        ws,
        b1=np.asarray(b1, np.float32),
        b2=np.asarray(b2, np.float32),
        b3rep=np.ascontiguousarray(
            np.broadcast_to(np.asarray(b3, np.float32), (128, F))
        ),
    )
    x = np.ascontiguousarray(np.asarray(x, np.float32))
    in_maps = []
    for c in range(NCORES):
        xs = x[c * BPC : (c + 1) * BPC]
        masked_t = np.ascontiguousarray(xs[:, 0::2].T)  # [F, BPC] f32
        m = dict(common, x=xs, mT=masked_t.astype(np.float16))
        if split:
            m["mTl"] = (masked_t - m["mT"].astype(np.float32)).astype(np.float16)
        in_maps.append(m)
    return in_maps


def kernel(x, W1, b1, W2, b2, W3, b3):
    from concourse.bass_utils import run_bass_kernel_spmd

    nc = _get(MODE)
    res = run_bass_kernel_spmd(
        nc, _in_maps(x, W1, b1, W2, b2, W3, b3), core_ids=list(range(NCORES))
    )
    return np.concatenate([res.results[c]["y"] for c in range(NCORES)], axis=0)
